# revision 1
# baseline (speedup 1.0000x reference)
"""Trainium2 Bass kernel for nn_ControlledConvEMAStabilizer.

Pipeline (per batch image, one NeuronCore each, batch-parallel over 8 cores):
  q = cat(backbone, z, mem_stab, mem_unstab)          # 160ch
  q = lrelu(conv3x3(q, w0) + b0)                      # -> 64ch
  q = lrelu(conv3x3(q, w1) + b1)                      # -> 64ch
  q = lrelu(conv3x3(q, w2) + b2)                      # -> 64ch
  head = conv3x3(q, w_last) + b_last                  # -> 288ch = 9 taps x 32ch
  eta  = softmax([head; 0]) over the 9+1 slots
  out  = sum_p unfold(mem_stab)[p] * eta[p] + eta[9] * z

Implementation notes:
  - Feature maps live in SBUF as zero-padded flat rows: image pixel (r,c) at
    column 129*(r+1)+1+c  (row stride 129, shared single pad column between
    rows, one pad row top/bottom).  Every 3x3 tap is then a pure column
    offset t = 129*dr + dc, so convs are PSUM-accumulated matmuls over
    shifted views (float32r -> full PE rate at N>=256).
  - K-stacking: each intermediate tensor is stored twice in one [128, NCOL]
    tile: partitions 0:64 = q, partitions 64:128 = q shifted by +129 (one
    image row).  A K=128 matmul then applies two vertical taps at once.
  - LeakyReLU: y = (x + b) + Relu(-0.99*(x + b)), via one ScalarE activation
    (scale=-0.99, bias=-0.99b) + one fused DVE scalar_tensor_tensor.
  - Tail fused per 3-row strip: conv_last (18 mm) -> Exp(+b_last) on ACT ->
    multiply with shifted mem_stab patches (DVE) -> partition-group sums via
    block-identity matmuls (PE) -> reciprocal_approx_fast -> out.
"""

import numpy as np
from contextlib import ExitStack

import concourse.bacc as bacc
import concourse.tile as tile
from concourse import mybir
from concourse.bass_utils import run_bass_kernel_spmd

F32 = mybir.dt.float32
F32R = mybir.dt.float32r
BF16 = mybir.dt.bfloat16
ALU = mybir.AluOpType
ACTF = mybir.ActivationFunctionType

H = 128
ST = 129                      # padded row stride
NCOL = ST * 130 + 2           # 16772 sbuf cols (incl 1 extra tail zero)
XCOL = NCOL                   # dram padded cols for xpad
MUCOL = NCOL + 2 * ST + 2     # mu needs reads up to +258 further
ROWS_PER_STRIP = 3
X_GROUP_STRIPS = 3            # conv0 input staging granularity (9 rows)

# taps in fusion/unfold order p = 3*kh + kw -> offset 129*(kh-1) + (kw-1)
P_TAPS = [ST * (kh - 1) + (kw - 1) for kh in range(3) for kw in range(3)]


def _j0(r0):
    return ST * (r0 + 1) + 1


def _strips():
    out = []
    r0 = 0
    while r0 < H:
        nr = min(ROWS_PER_STRIP, H - r0)
        out.append((r0, nr))
        r0 += nr
    return out


def _build_program(debug=False):
    nc = bacc.Bacc("TRN2", target_bir_lowering=False, debug=False)

    d_xpad = nc.dram_tensor("xpad", [128, XCOL], BF16, kind="ExternalInput")
    d_mupad = nc.dram_tensor("mupad", [32, MUCOL], BF16, kind="ExternalInput")
    d_w0c1 = nc.dram_tensor("w0c1", [128, 9 * 64], BF16, kind="ExternalInput")
    d_w0c2 = nc.dram_tensor("w0c2", [96, 3 * 64], BF16, kind="ExternalInput")
    d_w1P = nc.dram_tensor("w1P", [128, 3 * 64], BF16, kind="ExternalInput")
    d_w1S = nc.dram_tensor("w1S", [64, 3 * 64], BF16, kind="ExternalInput")
    d_w2P = nc.dram_tensor("w2P", [128, 3 * 64], BF16, kind="ExternalInput")
    d_w2S = nc.dram_tensor("w2S", [64, 3 * 64], BF16, kind="ExternalInput")
    d_wlP = nc.dram_tensor("wlP", [128, 3 * 288], BF16, kind="ExternalInput")
    d_wlS = nc.dram_tensor("wlS", [64, 3 * 288], BF16, kind="ExternalInput")
    d_b = nc.dram_tensor("bias", [64, 6], F32, kind="ExternalInput")  # b0,b0n,b1,b1n,b2,b2n
    d_blp = nc.dram_tensor("blp", [128, 3], F32, kind="ExternalInput")  # 288 perm bias, col-chunks
    d_eye = nc.dram_tensor("eye", [128, 32], BF16, kind="ExternalInput")
    d_out = nc.dram_tensor("out", [32, H, H], F32, kind="ExternalOutput")
    if debug:
        d_q1 = nc.dram_tensor("dbg_q1", [128, NCOL], F32, kind="ExternalOutput")
        d_q2 = nc.dram_tensor("dbg_q2", [128, NCOL], F32, kind="ExternalOutput")
        d_q3 = nc.dram_tensor("dbg_q3", [128, NCOL], F32, kind="ExternalOutput")

    strips = _strips()

    with tile.TileContext(nc) as tc, ExitStack() as ctx:
        wp = ctx.enter_context(tc.tile_pool(name="wp", bufs=1))
        big = ctx.enter_context(tc.tile_pool(name="big", bufs=1))
        xs = ctx.enter_context(tc.tile_pool(name="xs", bufs=2))
        sm = ctx.enter_context(tc.tile_pool(name="sm", bufs=3))
        fu = ctx.enter_context(tc.tile_pool(name="fu", bufs=2))
        pA = ctx.enter_context(tc.tile_pool(name="pA", bufs=2, space="PSUM"))
        pB = ctx.enter_context(tc.tile_pool(name="pB", bufs=2, space="PSUM"))
        pC = ctx.enter_context(tc.tile_pool(name="pC", bufs=2, space="PSUM"))
        pD = ctx.enter_context(tc.tile_pool(name="pD", bufs=2, space="PSUM"))

        # ---- weights / constants to SBUF ----
        w0c1 = wp.tile([128, 9 * 64], BF16)
        w0c2 = wp.tile([96, 3 * 64], BF16)
        w1P = wp.tile([128, 3 * 64], BF16)
        w1S = wp.tile([64, 3 * 64], BF16)
        w2P = wp.tile([128, 3 * 64], BF16)
        w2S = wp.tile([64, 3 * 64], BF16)
        wlP = wp.tile([128, 3 * 288], BF16)
        wlS = wp.tile([64, 3 * 288], BF16)
        bias = wp.tile([64, 6], F32)
        blp = wp.tile([128, 3], F32)
        eye = wp.tile([128, 32], BF16)
        for dst, src in ((w0c1, d_w0c1), (w0c2, d_w0c2), (w1P, d_w1P),
                         (w1S, d_w1S), (w2P, d_w2P), (w2S, d_w2S),
                         (wlP, d_wlP), (wlS, d_wlS), (eye, d_eye)):
            nc.sync.dma_start(out=dst[:], in_=src.ap())
        for dst, src in ((bias, d_b), (blp, d_blp)):
            nc.sync.dma_start(out=dst[:], in_=src.ap())

        def wslice(wt, i, m0, mw, step=64):
            # [K, mw] slice for matmul lhsT: tap/dc index i, out-ch offset m0
            return wt[:, i * step + m0: i * step + m0 + mw]

        def r_(t):
            return t

        # ---- big feature tiles (two slots: A holds q1 then q3, B holds q2) ----
        def new_q(tag):
            q = big.tile([128, NCOL], BF16, tag=tag)
            # zero the pad structure (lower half: head, inter-row cells, tail;
            # upper half: head cell + tail region never covered by upcopies)
            nc.gpsimd.memset(q[0:64, 0:130], 0.0)
            inter = q[0:64, 258:258 + 127 * ST].rearrange(
                "p (m s) -> p m s", s=ST)[:, :, 0:1]
            nc.gpsimd.memset(inter, 0.0)
            nc.gpsimd.memset(q[0:64, ST * 129:NCOL], 0.0)
            nc.gpsimd.memset(q[64:128, 0:1], 0.0)
            last_up = _j0(strips[-1][0]) - ST + strips[-1][1] * ST
            nc.gpsimd.memset(q[64:128, last_up:NCOL], 0.0)
            return q

        def evac_conv(ps, q, j0, nr, n, bcol):
            # leaky-relu from psum into q's valid cells + shifted upper copy
            rn = sm.tile([64, 3 * ST], F32, tag="rn")
            nc.scalar.activation(rn[:, 0:n], ps[:, 0:n], ACTF.Relu,
                                 bias=bias[:, bcol + 1:bcol + 2], scale=-0.99)
            src = ps[:, 0:n].rearrange("p (r c) -> p r c", c=ST)[:, :, 0:128]
            rnv = rn[:, 0:n].rearrange("p (r c) -> p r c", c=ST)[:, :, 0:128]
            dst = q[0:64, j0:j0 + n].rearrange("p (r c) -> p r c", c=ST)[:, :, 0:128]
            nc.vector.scalar_tensor_tensor(dst, src,
                                           bias[:, bcol:bcol + 1], rnv,
                                           op0=ALU.add, op1=ALU.add)
            # upper K-stack copy: up[j] = q[j+129] over this strip's window
            nc.sync.dma_start(out=q[64:128, j0 - ST:j0 - ST + n],
                              in_=q[0:64, j0:j0 + n])

        # ================= conv0 (streamed input strips) =================
        q1 = new_q("A")
        gi = 0
        while gi < len(strips):
            grp = strips[gi:gi + X_GROUP_STRIPS]
            r0g = grp[0][0]
            nrg = sum(nr for _, nr in grp)
            jg = _j0(r0g)
            win = ST * nrg + 260
            x1 = xs.tile([128, ST * 9 + 260], BF16, tag="x1")
            x2 = xs.tile([96, ST * 9 + 260], BF16, tag="x2")
            nc.sync.dma_start(out=x1[:, 0:win], in_=d_xpad.ap()[:, jg - 130:jg - 130 + win])
            for k in range(3):
                nc.sync.dma_start(
                    out=x2[32 * k:32 * k + 32, 0:win],
                    in_=d_mupad.ap()[:, jg - 130 + ST * k:jg - 130 + ST * k + win])
            for (r0, nr) in grp:
                j0 = _j0(r0)
                n = ST * nr
                loc = j0 - jg + 130
                ps = pA.tile([64, 3 * ST], F32, tag="pA")
                first = True
                for t, (dr, dc) in enumerate([(a, b) for a in (-1, 0, 1) for b in (-1, 0, 1)]):
                    o = loc + ST * dr + dc
                    nc.tensor.matmul(ps[:, 0:n], r_(wslice(w0c1, t, 0, 64)),
                                     r_(x1[:, o:o + n]), start=first, stop=False)
                    first = False
                for i, dc in enumerate((-1, 0, 1)):
                    o = loc - ST + dc
                    nc.tensor.matmul(ps[:, 0:n], r_(wslice(w0c2, i, 0, 64)),
                                     r_(x2[:, o:o + n]), start=False, stop=(i == 2))
                evac_conv(ps, q1, j0, nr, n, 0)
            gi += X_GROUP_STRIPS
        if debug:
            nc.sync.dma_start(out=d_q1.ap(), in_=q1[:])

        # ================= conv1 / conv2 =================
        def mid_conv(qin, qout, wP, wS, bcol):
            for (r0, nr) in strips:
                j0 = _j0(r0)
                n = ST * nr
                ps = pA.tile([64, 3 * ST], F32, tag="pA")
                for i, dc in enumerate((-1, 0, 1)):
                    o = j0 - ST + dc
                    nc.tensor.matmul(ps[:, 0:n], r_(wslice(wP, i, 0, 64)),
                                     r_(qin[0:128, o:o + n]), start=(i == 0), stop=False)
                for i, dc in enumerate((-1, 0, 1)):
                    o = j0 + ST + dc
                    nc.tensor.matmul(ps[:, 0:n], r_(wslice(wS, i, 0, 64)),
                                     r_(qin[0:64, o:o + n]), start=False, stop=(i == 2))
                evac_conv(ps, qout, j0, nr, n, bcol)

        q2 = new_q("B")
        mid_conv(q1, q2, w1P, w1S, 2)
        if debug:
            nc.sync.dma_start(out=d_q2.ap(), in_=q2[:])

        q3 = new_q("A")
        mid_conv(q2, q3, w2P, w2S, 4)
        if debug:
            nc.sync.dma_start(out=d_q3.ap(), in_=q3[:])

        # ================= conv_last + softmax + fusion =================
        for (r0, nr) in strips:
            j0 = _j0(r0)
            n = ST * nr
            ph = [pA.tile([128, 3 * ST], F32, tag="pA", name="ph0"),
                  pB.tile([128, 3 * ST], F32, tag="pB", name="ph1"),
                  pC.tile([32, 3 * ST], F32, tag="pC", name="ph2")]
            for ci, (m0, mw) in enumerate(((0, 128), (128, 128), (256, 32))):
                ps = ph[ci]
                for i, dc in enumerate((-1, 0, 1)):
                    o = j0 - ST + dc
                    nc.tensor.matmul(ps[:, 0:n], r_(wslice(wlP, i, m0, mw, 288)),
                                     r_(q3[0:128, o:o + n]), start=(i == 0), stop=False)
                for i, dc in enumerate((-1, 0, 1)):
                    o = j0 + ST + dc
                    nc.tensor.matmul(ps[:, 0:n], r_(wslice(wlS, i, m0, mw, 288)),
                                     r_(q3[0:64, o:o + n]), start=False, stop=(i == 2))
            # exp(head + b_last)
            ea = fu.tile([128, 3 * ST], BF16, tag="ea")
            eb = fu.tile([128, 3 * ST], BF16, tag="eb")
            ec = fu.tile([32, 3 * ST], BF16, tag="ec")
            nc.scalar.activation(ea[:, 0:n], ph[0][:, 0:n], ACTF.Exp, bias=blp[:, 0:1])
            nc.scalar.activation(eb[:, 0:n], ph[1][:, 0:n], ACTF.Exp, bias=blp[:, 1:2])
            nc.scalar.activation(ec[:, 0:n], ph[2][:, 0:n], ACTF.Exp, bias=blp[0:32, 2:3])
            # patch strips of mem_stab (xpad rows 96:128), z strip (rows 64:96)
            msa = fu.tile([128, 3 * ST], BF16, tag="msa")
            msb = fu.tile([128, 3 * ST], BF16, tag="msb")
            msc = fu.tile([32, 3 * ST], BF16, tag="msc")
            for g in range(4):
                nc.sync.dma_start(out=msa[32 * g:32 * g + 32, 0:n],
                                  in_=d_xpad.ap()[96:128, j0 + P_TAPS[g]:j0 + P_TAPS[g] + n])
                nc.sync.dma_start(out=msb[32 * g:32 * g + 32, 0:n],
                                  in_=d_xpad.ap()[96:128, j0 + P_TAPS[4 + g]:j0 + P_TAPS[4 + g] + n])
            nc.sync.dma_start(out=msc[:, 0:n],
                              in_=d_xpad.ap()[96:128, j0 + P_TAPS[8]:j0 + P_TAPS[8] + n])
            rhs3 = fu.tile([64, 3 * ST], BF16, tag="rhs3")
            nc.sync.dma_start(out=rhs3[32:64, 0:n], in_=d_xpad.ap()[64:96, j0:j0 + n])
            ta = fu.tile([128, 3 * ST], BF16, tag="ta")
            tb = fu.tile([128, 3 * ST], BF16, tag="tb")
            nc.vector.tensor_mul(ta[:, 0:n], ea[:, 0:n], msa[:, 0:n])
            nc.vector.tensor_mul(tb[:, 0:n], eb[:, 0:n], msb[:, 0:n])
            nc.vector.tensor_mul(rhs3[0:32, 0:n], ec[:, 0:n], msc[:, 0:n])
            # numerator (psum 0:32) and denominator (psum 32:64)
            nd = pD.tile([64, 3 * ST], F32, tag="pD")
            nc.tensor.matmul(nd[0:32, 0:n], r_(eye[:]), r_(ta[:, 0:n]), start=True, stop=False)
            nc.tensor.matmul(nd[0:32, 0:n], r_(eye[:]), r_(tb[:, 0:n]), start=False, stop=False)
            nc.tensor.matmul(nd[0:32, 0:n], r_(eye[0:64, :]), r_(rhs3[:, 0:n]), start=False, stop=True)
            nc.tensor.matmul(nd[32:64, 0:n], r_(eye[:]), r_(ea[:, 0:n]), start=True, stop=False)
            nc.tensor.matmul(nd[32:64, 0:n], r_(eye[:]), r_(eb[:, 0:n]), start=False, stop=False)
            nc.tensor.matmul(nd[32:64, 0:n], r_(eye[0:32, :]), r_(ec[:, 0:n]), start=False, stop=True)
            den = fu.tile([32, 3 * ST], F32, tag="den")
            rde = fu.tile([32, 3 * ST], F32, tag="rde")
            ost = fu.tile([32, 3 * ST], F32, tag="ost")
            nc.vector.tensor_scalar_add(den[:, 0:n], nd[32:64, 0:n], 1.0)
            nc.vector.reciprocal_approx_fast(rde[:, 0:n], den[:, 0:n])
            nc.vector.tensor_mul(ost[:, 0:n], nd[0:32, 0:n], rde[:, 0:n])
            src = ost[:, 0:n].rearrange("p (r c) -> p r c", c=ST)[:, :, 0:128]
            nc.sync.dma_start(out=d_out.ap()[:, r0:r0 + nr, :], in_=src)

    nc.compile()
    return nc


BF16_NP = mybir.dt.np(mybir.dt.bfloat16)


def _pad_rows(x, cols):
    # x: [C, 128, 128] -> zero-padded flat rows [C, cols], bf16
    c = x.shape[0]
    buf = np.zeros((c, cols), dtype=BF16_NP)
    buf[:, 130:130 + ST * 128].reshape(c, 128, ST)[:, :, 0:128] = x.astype(BF16_NP)
    return buf


def _prep_shared(w0, b0, w1, b1, w2, b2, w_last, b_last):
    f = np.float32
    w0t = np.transpose(np.asarray(w0, f), (1, 2, 3, 0))      # [160,3,3,64]
    w0c1 = np.ascontiguousarray(w0t[0:128].reshape(128, 9 * 64))
    w0c2 = np.ascontiguousarray(
        np.transpose(w0t[128:160], (1, 0, 2, 3)).reshape(96, 3 * 64))
    def mid(w):
        wt = np.transpose(np.asarray(w, f), (1, 2, 3, 0))    # [64,3,3,64]
        wP = np.ascontiguousarray(
            np.concatenate([wt[:, 0], wt[:, 1]], 0).reshape(128, 3 * 64))
        wS = np.ascontiguousarray(wt[:, 2].reshape(64, 3 * 64))
        return wP, wS
    w1P, w1S = mid(w1)
    w2P, w2S = mid(w2)
    perm = np.array([(pp % 32) * 9 + pp // 32 for pp in range(288)])
    wl2 = np.asarray(w_last, f)[perm]                        # [288,64,3,3] p-major
    wlt = np.transpose(wl2, (1, 2, 3, 0))                    # [64,3,3,288]
    wlP = np.ascontiguousarray(
        np.concatenate([wlt[:, 0], wlt[:, 1]], 0).reshape(128, 3 * 288))
    wlS = np.ascontiguousarray(wlt[:, 2].reshape(64, 3 * 288))
    bias = np.stack([np.asarray(b0, f), -0.99 * np.asarray(b0, f),
                     np.asarray(b1, f), -0.99 * np.asarray(b1, f),
                     np.asarray(b2, f), -0.99 * np.asarray(b2, f)], axis=1)
    blp_flat = np.asarray(b_last, f)[perm]
    blp = np.zeros((128, 3), f)
    blp[:, 0] = blp_flat[0:128]
    blp[:, 1] = blp_flat[128:256]
    blp[0:32, 2] = blp_flat[256:288]
    eye = np.tile(np.eye(32, dtype=f), (4, 1))
    out = dict(w0c1=w0c1, w0c2=w0c2, w1P=w1P, w1S=w1S, w2P=w2P, w2S=w2S,
               wlP=wlP, wlS=wlS, eye=eye)
    out = {k: v.astype(BF16_NP) for k, v in out.items()}
    out["bias"] = np.ascontiguousarray(bias)
    out["blp"] = blp
    return out


_NC_CACHE = {}


def _get_nc(debug=False):
    if debug not in _NC_CACHE:
        _NC_CACHE[debug] = _build_program(debug)
    return _NC_CACHE[debug]


def make_in_maps(z, backbone, mem_stab, mem_unstab, shared):
    f = np.float32
    z = np.asarray(z, f); backbone = np.asarray(backbone, f)
    ms = np.asarray(mem_stab, f); mu = np.asarray(mem_unstab, f)
    maps = []
    for b in range(z.shape[0]):
        x160 = np.concatenate([backbone[b], z[b], ms[b]], axis=0)  # [128,...]
        maps.append(dict(xpad=_pad_rows(x160, XCOL),
                         mupad=_pad_rows(mu[b], MUCOL), **shared))
    return maps


def kernel(z, backbone, mem_stab, mem_unstab, w0, b0, w1, b1, w2, b2,
           w_last, b_last, fusion_kernel_size):
    assert int(fusion_kernel_size) == 3
    shared = _prep_shared(w0, b0, w1, b1, w2, b2, w_last, b_last)
    in_maps = make_in_maps(z, backbone, mem_stab, mem_unstab, shared)
    nc = _get_nc()
    res = run_bass_kernel_spmd(nc, in_maps, core_ids=list(range(len(in_maps))))
    out = np.stack([r["out"] for r in res.results], axis=0)
    return out.astype(np.float32)



# revision 3
# speedup vs baseline: 1.2041x; 1.2041x over previous
"""Trainium2 Bass kernel for nn_ControlledConvEMAStabilizer.

Pipeline (per batch image, one NeuronCore each, batch-parallel over 8 cores):
  q = cat(backbone, z, mem_stab, mem_unstab)          # 160ch
  q = lrelu(conv3x3(q, w0) + b0)                      # -> 64ch
  q = lrelu(conv3x3(q, w1) + b1)                      # -> 64ch
  q = lrelu(conv3x3(q, w2) + b2)                      # -> 64ch
  head = conv3x3(q, w_last) + b_last                  # -> 288ch = 9 taps x 32ch
  eta  = softmax([head; 0]) over the 9+1 slots
  out  = sum_p unfold(mem_stab)[p] * eta[p] + eta[9] * z

Implementation notes:
  - Feature maps live in SBUF as zero-padded flat rows: image pixel (r,c) at
    column 129*(r+1)+1+c  (row stride 129, shared single pad column between
    rows, one pad row top/bottom).  Every 3x3 tap is then a pure column
    offset t = 129*dr + dc, so convs are PSUM-accumulated matmuls over
    shifted views.
  - K-stacking: each intermediate tensor is stored twice in one [128, NCOL]
    tile: partitions 0:64 = q, partitions 64:128 = q shifted by +129 (one
    image row).  A K=128 matmul then applies two vertical taps at once.
  - LeakyReLU: y = (x + b) + Relu(-0.99*(x + b)), via one ScalarE activation
    (scale=-0.99, bias=-0.99b) + one fused DVE scalar_tensor_tensor.
  - Strip-wavefront schedule: a single loop where iteration `it` issues
    conv0(strip it), conv1(it-2), conv2(it-4), conv_last+exp+patch-mul(it-6)
    and fusion+output(it-7).  Every PE instruction's operands are produced
    a full iteration earlier, so the tensor engine runs one dense
    back-to-back matmul stream with no phase boundaries or per-strip
    dependency bubbles (which would re-throttle the PE clock via HAM).
  - Each stage has its own PSUM bank (conv0 double-buffered, rest single).
"""

import numpy as np
from contextlib import ExitStack

import concourse.bacc as bacc
import concourse.tile as tile
from concourse import mybir
from concourse.bass_utils import run_bass_kernel_spmd

F32 = mybir.dt.float32
BF16 = mybir.dt.bfloat16
ALU = mybir.AluOpType
ACTF = mybir.ActivationFunctionType

H = 128
ST = 129                      # padded row stride
NCOL = ST * 130 + 2           # 16772 sbuf cols (incl 1 extra tail zero)
XCOL = NCOL                   # dram padded cols for xpad
MUCOL = NCOL + 2 * ST + 2     # mu needs reads up to +258 further
ROWS_PER_STRIP = 3
X_GROUP_STRIPS = 3            # conv0 input staging granularity (9 rows)

# taps in fusion/unfold order p = 3*kh + kw -> offset 129*(kh-1) + (kw-1)
P_TAPS = [ST * (kh - 1) + (kw - 1) for kh in range(3) for kw in range(3)]


def _j0(r0):
    return ST * (r0 + 1) + 1


def _strips():
    out = []
    r0 = 0
    while r0 < H:
        nr = min(ROWS_PER_STRIP, H - r0)
        out.append((r0, nr))
        r0 += nr
    return out


def _build_program(debug=False):
    nc = bacc.Bacc("TRN2", target_bir_lowering=False, debug=False)

    d_xpad = nc.dram_tensor("xpad", [128, XCOL], BF16, kind="ExternalInput")
    d_mupad = nc.dram_tensor("mupad", [32, MUCOL], BF16, kind="ExternalInput")
    d_w0c1 = nc.dram_tensor("w0c1", [128, 9 * 64], BF16, kind="ExternalInput")
    d_w0c2 = nc.dram_tensor("w0c2", [96, 3 * 64], BF16, kind="ExternalInput")
    d_w1P = nc.dram_tensor("w1P", [128, 3 * 64], BF16, kind="ExternalInput")
    d_w1S = nc.dram_tensor("w1S", [64, 3 * 64], BF16, kind="ExternalInput")
    d_w2P = nc.dram_tensor("w2P", [128, 3 * 64], BF16, kind="ExternalInput")
    d_w2S = nc.dram_tensor("w2S", [64, 3 * 64], BF16, kind="ExternalInput")
    d_wlP = nc.dram_tensor("wlP", [128, 3 * 288], BF16, kind="ExternalInput")
    d_wlS = nc.dram_tensor("wlS", [64, 3 * 288], BF16, kind="ExternalInput")
    d_b = nc.dram_tensor("bias", [64, 6], F32, kind="ExternalInput")  # b0,b0n,b1,b1n,b2,b2n
    d_blp = nc.dram_tensor("blp", [128, 3], F32, kind="ExternalInput")  # 288 perm bias, col-chunks
    d_eye = nc.dram_tensor("eye", [128, 32], BF16, kind="ExternalInput")
    d_out = nc.dram_tensor("out", [32, H, H], F32, kind="ExternalOutput")

    strips = _strips()
    NS = len(strips)

    with tile.TileContext(nc) as tc, ExitStack() as ctx:
        wp = ctx.enter_context(tc.tile_pool(name="wp", bufs=1))
        big = ctx.enter_context(tc.tile_pool(name="big", bufs=1))
        xs = ctx.enter_context(tc.tile_pool(name="xs", bufs=2))
        sm = ctx.enter_context(tc.tile_pool(name="sm", bufs=3))
        fu = ctx.enter_context(tc.tile_pool(name="fu", bufs=2))
        p0 = ctx.enter_context(tc.tile_pool(name="p0", bufs=2, space="PSUM"))
        pp = ctx.enter_context(tc.tile_pool(name="pp", bufs=1, space="PSUM"))

        # ---- weights / constants to SBUF ----
        w0c1 = wp.tile([128, 9 * 64], BF16)
        w0c2 = wp.tile([96, 3 * 64], BF16)
        w1P = wp.tile([128, 3 * 64], BF16)
        w1S = wp.tile([64, 3 * 64], BF16)
        w2P = wp.tile([128, 3 * 64], BF16)
        w2S = wp.tile([64, 3 * 64], BF16)
        wlP = wp.tile([128, 3 * 288], BF16)
        wlS = wp.tile([64, 3 * 288], BF16)
        bias = wp.tile([64, 6], F32)
        blp = wp.tile([128, 3], F32)
        eye = wp.tile([128, 32], BF16)
        for dst, src in ((w0c1, d_w0c1), (w0c2, d_w0c2), (w1P, d_w1P),
                         (w1S, d_w1S), (w2P, d_w2P), (w2S, d_w2S),
                         (wlP, d_wlP), (wlS, d_wlS), (eye, d_eye)):
            nc.sync.dma_start(out=dst[:], in_=src.ap())
        for dst, src in ((bias, d_b), (blp, d_blp)):
            nc.sync.dma_start(out=dst[:], in_=src.ap())

        def wslice(wt, i, m0, mw, step=64):
            # [K, mw] slice for matmul lhsT: tap/dc index i, out-ch offset m0
            return wt[:, i * step + m0: i * step + m0 + mw]

        # ---- big feature tiles (q1, q2, q3: pad structure zeroed once) ----
        def new_q(tag):
            q = big.tile([128, NCOL], BF16, tag=tag)
            # zero the pad structure (lower half: head, inter-row cells, tail;
            # upper half: head cell + tail region never covered by upcopies)
            nc.gpsimd.memset(q[0:64, 0:130], 0.0)
            inter = q[0:64, 258:258 + 127 * ST].rearrange(
                "p (m s) -> p m s", s=ST)[:, :, 0:1]
            nc.gpsimd.memset(inter, 0.0)
            nc.gpsimd.memset(q[0:64, ST * 129:NCOL], 0.0)
            nc.gpsimd.memset(q[64:128, 0:1], 0.0)
            last_up = _j0(strips[-1][0]) - ST + strips[-1][1] * ST
            nc.gpsimd.memset(q[64:128, last_up:NCOL], 0.0)
            return q

        q1 = new_q("A")
        q2 = new_q("B")
        q3 = new_q("C")

        def evac_conv(ps, q, j0, nr, n, bcol):
            # leaky-relu from psum into q's valid cells + shifted upper copy
            rn = sm.tile([64, 3 * ST], F32, tag="rn")
            nc.scalar.activation(rn[:, 0:n], ps[:, 0:n], ACTF.Relu,
                                 bias=bias[:, bcol + 1:bcol + 2], scale=-0.99)
            src = ps[:, 0:n].rearrange("p (r c) -> p r c", c=ST)[:, :, 0:128]
            rnv = rn[:, 0:n].rearrange("p (r c) -> p r c", c=ST)[:, :, 0:128]
            dst = q[0:64, j0:j0 + n].rearrange("p (r c) -> p r c", c=ST)[:, :, 0:128]
            nc.vector.scalar_tensor_tensor(dst, src,
                                           bias[:, bcol:bcol + 1], rnv,
                                           op0=ALU.add, op1=ALU.add)
            # upper K-stack copy: up[j] = q[j+129] over this strip's window
            nc.sync.dma_start(out=q[64:128, j0 - ST:j0 - ST + n],
                              in_=q[0:64, j0:j0 + n])

        # ---- conv0 input staging (groups of 3 strips) ----
        gstate = {}

        def stage_group(g):
            grp = strips[3 * g:3 * g + X_GROUP_STRIPS]
            r0g = grp[0][0]
            nrg = sum(nr for _, nr in grp)
            jg = _j0(r0g)
            win = ST * nrg + 260
            x1 = xs.tile([128, ST * 9 + 260], BF16, tag="x1")
            x2 = xs.tile([96, ST * 9 + 260], BF16, tag="x2")
            nc.sync.dma_start(out=x1[:, 0:win], in_=d_xpad.ap()[:, jg - 130:jg - 130 + win])
            for k in range(3):
                nc.sync.dma_start(
                    out=x2[32 * k:32 * k + 32, 0:win],
                    in_=d_mupad.ap()[:, jg - 130 + ST * k:jg - 130 + ST * k + win])
            gstate[g] = (x1, x2, jg)

        def conv0_strip(s):
            r0, nr = strips[s]
            j0 = _j0(r0)
            n = ST * nr
            x1, x2, jg = gstate[s // 3]
            loc = j0 - jg + 130
            ps = p0.tile([64, 3 * ST], F32, tag="c0")
            first = True
            for t, (dr, dc) in enumerate([(a, b) for a in (-1, 0, 1) for b in (-1, 0, 1)]):
                o = loc + ST * dr + dc
                nc.tensor.matmul(ps[:, 0:n], wslice(w0c1, t, 0, 64),
                                 x1[:, o:o + n], start=first, stop=False)
                first = False
            for i, dc in enumerate((-1, 0, 1)):
                o = loc - ST + dc
                nc.tensor.matmul(ps[:, 0:n], wslice(w0c2, i, 0, 64),
                                 x2[:, o:o + n], start=False, stop=(i == 2))
            evac_conv(ps, q1, j0, nr, n, 0)

        def conv_mid(s, qin, qout, wP, wS, bcol, tag):
            r0, nr = strips[s]
            j0 = _j0(r0)
            n = ST * nr
            ps = pp.tile([64, 3 * ST], F32, tag=tag)
            for i, dc in enumerate((-1, 0, 1)):
                o = j0 - ST + dc
                nc.tensor.matmul(ps[:, 0:n], wslice(wP, i, 0, 64),
                                 qin[0:128, o:o + n], start=(i == 0), stop=False)
            for i, dc in enumerate((-1, 0, 1)):
                o = j0 + ST + dc
                nc.tensor.matmul(ps[:, 0:n], wslice(wS, i, 0, 64),
                                 qin[0:64, o:o + n], start=False, stop=(i == 2))
            evac_conv(ps, qout, j0, nr, n, bcol)

        def tail_head(s):
            # conv_last (18 mm) -> Exp(+b_last) -> patch loads + multiplies
            r0, nr = strips[s]
            j0 = _j0(r0)
            n = ST * nr
            ph = [pp.tile([128, 3 * ST], F32, tag="h0", name="ph0"),
                  pp.tile([128, 3 * ST], F32, tag="h1", name="ph1"),
                  pp.tile([32, 3 * ST], F32, tag="h2", name="ph2")]
            for ci, (m0, mw) in enumerate(((0, 128), (128, 128), (256, 32))):
                ps = ph[ci]
                for i, dc in enumerate((-1, 0, 1)):
                    o = j0 - ST + dc
                    nc.tensor.matmul(ps[:, 0:n], wslice(wlP, i, m0, mw, 288),
                                     q3[0:128, o:o + n], start=(i == 0), stop=False)
                for i, dc in enumerate((-1, 0, 1)):
                    o = j0 + ST + dc
                    nc.tensor.matmul(ps[:, 0:n], wslice(wlS, i, m0, mw, 288),
                                     q3[0:64, o:o + n], start=False, stop=(i == 2))
            # exp(head + b_last)
            ea = fu.tile([128, 3 * ST], BF16, tag="ea")
            eb = fu.tile([128, 3 * ST], BF16, tag="eb")
            ec = fu.tile([32, 3 * ST], BF16, tag="ec")
            nc.scalar.activation(ea[:, 0:n], ph[0][:, 0:n], ACTF.Exp, bias=blp[:, 0:1])
            nc.scalar.activation(eb[:, 0:n], ph[1][:, 0:n], ACTF.Exp, bias=blp[:, 1:2])
            nc.scalar.activation(ec[:, 0:n], ph[2][:, 0:n], ACTF.Exp, bias=blp[0:32, 2:3])
            # patch strips of mem_stab (xpad rows 96:128), z strip (rows 64:96)
            msa = fu.tile([128, 3 * ST], BF16, tag="msa")
            msb = fu.tile([128, 3 * ST], BF16, tag="msb")
            msc = fu.tile([32, 3 * ST], BF16, tag="msc")
            for g in range(4):
                nc.sync.dma_start(out=msa[32 * g:32 * g + 32, 0:n],
                                  in_=d_xpad.ap()[96:128, j0 + P_TAPS[g]:j0 + P_TAPS[g] + n])
                nc.sync.dma_start(out=msb[32 * g:32 * g + 32, 0:n],
                                  in_=d_xpad.ap()[96:128, j0 + P_TAPS[4 + g]:j0 + P_TAPS[4 + g] + n])
            nc.sync.dma_start(out=msc[:, 0:n],
                              in_=d_xpad.ap()[96:128, j0 + P_TAPS[8]:j0 + P_TAPS[8] + n])
            rhs3 = fu.tile([64, 3 * ST], BF16, tag="rhs3")
            nc.sync.dma_start(out=rhs3[32:64, 0:n], in_=d_xpad.ap()[64:96, j0:j0 + n])
            ta = fu.tile([128, 3 * ST], BF16, tag="ta")
            tb = fu.tile([128, 3 * ST], BF16, tag="tb")
            nc.vector.tensor_mul(ta[:, 0:n], ea[:, 0:n], msa[:, 0:n])
            nc.vector.tensor_mul(tb[:, 0:n], eb[:, 0:n], msb[:, 0:n])
            nc.vector.tensor_mul(rhs3[0:32, 0:n], ec[:, 0:n], msc[:, 0:n])
            return dict(ea=ea, eb=eb, ec=ec, ta=ta, tb=tb, rhs3=rhs3, n=n,
                        r0=r0, nr=nr, j0=j0)

        def tail_fuse(st):
            # numerator (psum 0:32) and denominator (psum 32:64) -> output
            n, r0, nr = st["n"], st["r0"], st["nr"]
            ea, eb, ec = st["ea"], st["eb"], st["ec"]
            ta, tb, rhs3 = st["ta"], st["tb"], st["rhs3"]
            nd = pp.tile([64, 3 * ST], F32, tag="nd")
            nc.tensor.matmul(nd[0:32, 0:n], eye[:], ta[:, 0:n], start=True, stop=False)
            nc.tensor.matmul(nd[0:32, 0:n], eye[:], tb[:, 0:n], start=False, stop=False)
            nc.tensor.matmul(nd[0:32, 0:n], eye[0:64, :], rhs3[:, 0:n], start=False, stop=True)
            nc.tensor.matmul(nd[32:64, 0:n], eye[:], ea[:, 0:n], start=True, stop=False)
            nc.tensor.matmul(nd[32:64, 0:n], eye[:], eb[:, 0:n], start=False, stop=False)
            nc.tensor.matmul(nd[32:64, 0:n], eye[0:32, :], ec[:, 0:n], start=False, stop=True)
            den = fu.tile([32, 3 * ST], F32, tag="den")
            rde = fu.tile([32, 3 * ST], F32, tag="rde")
            ost = fu.tile([32, 3 * ST], F32, tag="ost")
            nc.vector.tensor_scalar_add(den[:, 0:n], nd[32:64, 0:n], 1.0)
            nc.vector.reciprocal_approx_fast(rde[:, 0:n], den[:, 0:n])
            nc.vector.tensor_mul(ost[:, 0:n], nd[0:32, 0:n], rde[:, 0:n])
            src = ost[:, 0:n].rearrange("p (r c) -> p r c", c=ST)[:, :, 0:128]
            nc.sync.dma_start(out=d_out.ap()[:, r0:r0 + nr, :], in_=src)

        # ---- wavefront main loop ----
        stage_group(0)
        tstate = {}
        for it in range(NS + 7):
            if it < NS and it % X_GROUP_STRIPS == 0:
                gnext = it // X_GROUP_STRIPS + 1
                if 3 * gnext < NS:
                    stage_group(gnext)
            if it < NS:
                conv0_strip(it)
            if 0 <= it - 2 < NS:
                conv_mid(it - 2, q1, q2, w1P, w1S, 2, "c1")
            if 0 <= it - 4 < NS:
                conv_mid(it - 4, q2, q3, w2P, w2S, 4, "c2")
            if 0 <= it - 6 < NS:
                tstate[it - 6] = tail_head(it - 6)
            if 0 <= it - 7 < NS:
                tail_fuse(tstate.pop(it - 7))

    nc.compile()
    return nc


BF16_NP = mybir.dt.np(mybir.dt.bfloat16)


def _pad_rows(x, cols):
    # x: [C, 128, 128] -> zero-padded flat rows [C, cols], bf16
    c = x.shape[0]
    buf = np.zeros((c, cols), dtype=BF16_NP)
    buf[:, 130:130 + ST * 128].reshape(c, 128, ST)[:, :, 0:128] = x.astype(BF16_NP)
    return buf


def _prep_shared(w0, b0, w1, b1, w2, b2, w_last, b_last):
    f = np.float32
    w0t = np.transpose(np.asarray(w0, f), (1, 2, 3, 0))      # [160,3,3,64]
    w0c1 = np.ascontiguousarray(w0t[0:128].reshape(128, 9 * 64))
    w0c2 = np.ascontiguousarray(
        np.transpose(w0t[128:160], (1, 0, 2, 3)).reshape(96, 3 * 64))
    def mid(w):
        wt = np.transpose(np.asarray(w, f), (1, 2, 3, 0))    # [64,3,3,64]
        wP = np.ascontiguousarray(
            np.concatenate([wt[:, 0], wt[:, 1]], 0).reshape(128, 3 * 64))
        wS = np.ascontiguousarray(wt[:, 2].reshape(64, 3 * 64))
        return wP, wS
    w1P, w1S = mid(w1)
    w2P, w2S = mid(w2)
    perm = np.array([(pp % 32) * 9 + pp // 32 for pp in range(288)])
    wl2 = np.asarray(w_last, f)[perm]                        # [288,64,3,3] p-major
    wlt = np.transpose(wl2, (1, 2, 3, 0))                    # [64,3,3,288]
    wlP = np.ascontiguousarray(
        np.concatenate([wlt[:, 0], wlt[:, 1]], 0).reshape(128, 3 * 288))
    wlS = np.ascontiguousarray(wlt[:, 2].reshape(64, 3 * 288))
    bias = np.stack([np.asarray(b0, f), -0.99 * np.asarray(b0, f),
                     np.asarray(b1, f), -0.99 * np.asarray(b1, f),
                     np.asarray(b2, f), -0.99 * np.asarray(b2, f)], axis=1)
    blp_flat = np.asarray(b_last, f)[perm]
    blp = np.zeros((128, 3), f)
    blp[:, 0] = blp_flat[0:128]
    blp[:, 1] = blp_flat[128:256]
    blp[0:32, 2] = blp_flat[256:288]
    eye = np.tile(np.eye(32, dtype=f), (4, 1))
    out = dict(w0c1=w0c1, w0c2=w0c2, w1P=w1P, w1S=w1S, w2P=w2P, w2S=w2S,
               wlP=wlP, wlS=wlS, eye=eye)
    out = {k: v.astype(BF16_NP) for k, v in out.items()}
    out["bias"] = np.ascontiguousarray(bias)
    out["blp"] = blp
    return out


_NC_CACHE = {}


def _get_nc(debug=False):
    if debug not in _NC_CACHE:
        _NC_CACHE[debug] = _build_program(debug)
    return _NC_CACHE[debug]


def make_in_maps(z, backbone, mem_stab, mem_unstab, shared):
    f = np.float32
    z = np.asarray(z, f); backbone = np.asarray(backbone, f)
    ms = np.asarray(mem_stab, f); mu = np.asarray(mem_unstab, f)
    maps = []
    for b in range(z.shape[0]):
        x160 = np.concatenate([backbone[b], z[b], ms[b]], axis=0)  # [128,...]
        maps.append(dict(xpad=_pad_rows(x160, XCOL),
                         mupad=_pad_rows(mu[b], MUCOL), **shared))
    return maps


def kernel(z, backbone, mem_stab, mem_unstab, w0, b0, w1, b1, w2, b2,
           w_last, b_last, fusion_kernel_size):
    assert int(fusion_kernel_size) == 3
    shared = _prep_shared(w0, b0, w1, b1, w2, b2, w_last, b_last)
    in_maps = make_in_maps(z, backbone, mem_stab, mem_unstab, shared)
    nc = _get_nc()
    res = run_bass_kernel_spmd(nc, in_maps, core_ids=list(range(len(in_maps))))
    out = np.stack([r["out"] for r in res.results], axis=0)
    return out.astype(np.float32)
